# revision 14
# baseline (speedup 1.0000x reference)
"""DeepSeek-V2 normal MoE layer on 8 Trainium2 NeuronCores.

Expert-parallel sharding: core e holds expert e's weights (cast to bf16).
The router (tiny [T,E] matmul + softmax + top-k) runs on the host in fp32 —
this is the dispatch step of the sharding layer: it decides which token rows
are copied to which core. Each core receives its routed tokens (bf16,
host-packed so every DMA moves contiguous partition rows) plus a broadcast
row of the renormalized top-k combine weights. On device, each core computes
the gated-SiLU expert MLP for its tokens (three 2048/1408-contraction matmul
phases in bf16 with fp32 PSUM accumulation, feature-major layout so no
on-device transposes are needed), applies the combine weight in the fp32
output epilogue, and the host scatter-adds the per-expert outputs back into
the full [T, H] result.

Weights are pre-tiled on the host into [n_tiles, 128, contract*128] blocks
so every DMA moves 4 KiB contiguous per partition (vs 256 B chunks when
slicing the natural [H, I] layout — measured 2x DMA throughput difference).

Per-core capacity C = max tokens routed to any expert (rounded up to 2);
pad token columns are zero and carry combine-weight 0, so their
contribution is exactly zero.
"""

import numpy as np
import ml_dtypes


def _ensure_ntff_hook():
    """This image's antenv package lacks axon_hooks, but concourse's
    run_bass_kernel_spmd unconditionally imports it when BASS_TRACE is set.
    Provide the module (and the ctypes NTFF hook from trn_agent_boot, when
    available) so tracing works instead of crashing. Idempotent; never
    overwrites an existing module."""
    import sys
    import types
    try:
        import antenv  # noqa: F401
    except ImportError:
        return
    if "antenv.axon_hooks" in sys.modules:
        return
    try:
        import antenv.axon_hooks  # noqa: F401
        return
    except ImportError:
        pass
    mod = types.ModuleType("antenv.axon_hooks")
    holder = {"h": None}
    mod.set_axon_ntff_profile_hook = lambda h: holder.__setitem__("h", h)
    mod.get_axon_ntff_profile_hook = lambda: holder.get("h")
    sys.modules["antenv.axon_hooks"] = mod
    import antenv as _a
    _a.axon_hooks = mod
    try:
        from trn_agent_boot.trn_boot import _ntff_profile_via_ctypes
        hook = _ntff_profile_via_ctypes("/opt/axon/libaxon_pjrt.so")
        if hook is not None:
            mod.set_axon_ntff_profile_hook(hook)
    except Exception:
        pass


_ensure_ntff_hook()

H = 2048
I_DIM = 1408
E = 8
P = 128
HT = H // P      # 16
IT = I_DIM // P  # 11

_compiled = {}
last_results = None


def _chunks(C):
    """Token-column chunks of <=512 (one PSUM bank / max moving free dim).
    Near-equal sizes with every chunk >=128 where possible: a matmul
    narrower than ~60 cycles pays the NX-dispatch floor (~25ns at 2.4GHz
    regardless of width), so (426,128) beats (512,42) by ~2.5us/kernel."""
    n = (C + 511) // 512
    if n == 1:
        return [(0, C)]
    out = []
    s = 0
    for i in range(n):
        w = C // n + (1 if i < C % n else 0)
        if i == n - 1:
            w = C - s
        out.append((s, w))
        s += w
    return out


def _build(C):
    import concourse.bacc as bacc
    import concourse.mybir as mybir
    import concourse.tile as tile

    dt = mybir.dt
    nc = bacc.Bacc("TRN2", target_bir_lowering=False)
    # Pre-tiled weight layouts: wg/wu [IT, 128, HT*128], wd [HT, 128, IT*128].
    # Block [t, p, k*128+c] = W[k*128+p, t*128+c] of the natural layout, i.e.
    # partition p of block t holds that block's full contraction row,
    # contiguous in DRAM.
    xg = nc.dram_tensor("xg", [P, HT * C], dt.bfloat16, kind="ExternalInput")
    wt = nc.dram_tensor("wt", [P, C], dt.float32, kind="ExternalInput")
    wg = nc.dram_tensor("wg", [IT, P, HT * P], dt.bfloat16, kind="ExternalInput")
    wu = nc.dram_tensor("wu", [IT, P, HT * P], dt.bfloat16, kind="ExternalInput")
    wd = nc.dram_tensor("wd", [HT, P, IT * P], dt.bfloat16, kind="ExternalInput")
    yt = nc.dram_tensor("yt", [H, C], dt.float32, kind="ExternalOutput")

    ch = _chunks(C)

    with tile.TileContext(nc) as tc:
        with (
            tc.tile_pool(name="xpool", bufs=1) as xpool,
            tc.tile_pool(name="apool", bufs=1) as apool,
            tc.tile_pool(name="wpool", bufs=3) as wpool,
            tc.tile_pool(name="wdpool", bufs=4) as wdpool,
            tc.tile_pool(name="spool", bufs=2) as spool,
            tc.tile_pool(name="ypool", bufs=3) as ypool,
        ):
            from concourse.tile_rust import add_dep_helper

            def load_wd(t):
                w_t = wdpool.tile([P, IT, P], dt.bfloat16, name="wd", tag="wd")
                ins = nc.gpsimd.dma_start(out=w_t[:], in_=wd[t, :, :])
                return w_t, ins

            # The DMA hardware queues are FIFO per queue: whatever triggers
            # first transfers first, and phase-1's front is delivery-bound
            # (~355 GB/s). So the trigger order must equal first-use order:
            # the 8 xg octets on Sync and it=0's wg/wu halves on GpSimd,
            # interleaved to match it=0's per-h g/u consumption. Every LATER
            # weight load is ring-paced (bufs=1 tag rings): iteration it's
            # half can only trigger once it-1's same-ring half has been fully
            # consumed — a real WAR dependency the scheduler must honor,
            # which pins all bulk weight traffic behind the whole xg block
            # in the queues.
            HO = HT // 8
            xq_t, xq_ins = [], []

            def load_xq(q):
                t = xpool.tile([P, HO * C], dt.bfloat16, name=f"xq{q}", tag=f"xq{q}")
                ins = nc.sync.dma_start(out=t[:], in_=xg[:, q * HO * C:(q + 1) * HO * C])
                xq_t.append(t)
                xq_ins.append(ins)

            def load_w_half(src, t, half, tag):
                w_t = wpool.tile([P, HT // 2, P], dt.bfloat16, name=f"{tag}{t}",
                                 tag=tag, bufs=1)
                ins = nc.gpsimd.dma_start(
                    out=w_t[:], in_=src[t, :, half * (HT // 2) * P:
                                        (half + 1) * (HT // 2) * P])
                return w_t, ins

            with tc.high_priority():
                load_xq(0)
                wgA, _ = load_w_half(wg, 0, 0, "wgA")
                load_xq(1)
                wuA, _ = load_w_half(wu, 0, 0, "wuA")
                load_xq(2)
                load_xq(3)
                wgB, _ = load_w_half(wg, 0, 1, "wgB")
                load_xq(4)
                wuB, _ = load_w_half(wu, 0, 1, "wuB")
                for q in range(5, 8):
                    load_xq(q)
            wb = xpool.tile([P, C], dt.float32, name="wb", tag="wb")
            wb_ins = nc.sync.dma_start(out=wb[:], in_=wt[:, :])
            # ordering-only edge: wb's trigger sits behind xq7's in the Sync
            # stream so its packets cannot cut ahead of token data
            add_dep_helper(wb_ins.ins, xq_ins[7].ins, sync=False)
            xg_t = [xq_t[h // HO][:, (h % HO) * C:(h % HO + 1) * C]
                    for h in range(HT)]

            # PE warm-up while token DMAs stream: ~6us of tiny matmuls on a
            # zeroed scratch tile releases the HAM clock gate (1.2 -> 2.4 GHz
            # takes ~3.4us of sustained PE activity) before real work lands.
            warm = spool.tile([P, 64], dt.bfloat16, name="warm", tag="warm")
            nc.vector.memset(warm[:], 0.0)

            # Phase 1: A[i, t] = silu(G) * U, feature-major, per 128-row i-tile.
            HH = HT // 2
            a_t = []
            anchor_ins = None
            with tc.tile_pool(name="pp1", bufs=2, space="PSUM") as pp1:
                for it in range(IT):
                    if it != 0:
                        # ring-paced (bufs=1): each trigger waits for it-1's
                        # same-ring half to be consumed, so weight prefetch
                        # self-paces ~one iteration ahead of the PE
                        wgA, a_ins = load_w_half(wg, it, 0, "wgA")
                        wuA, _ = load_w_half(wu, it, 0, "wuA")
                        wgB, _ = load_w_half(wg, it, 1, "wgB")
                        wuB, _ = load_w_half(wu, it, 1, "wuB")
                        if it == 5:
                            anchor_ins = a_ins
                    pgs = [pp1.tile([P, w], dt.float32, name=f"pg{ci}", tag=f"pg{ci}",
                                    bufs=2 if ci == 0 else 1)
                           for ci, (s, w) in enumerate(ch)]
                    pus = [pp1.tile([P, w], dt.float32, name=f"pu{ci}", tag=f"pu{ci}",
                                    bufs=2 if ci == 0 else 1)
                           for ci, (s, w) in enumerate(ch)]
                    if it == 0:
                        for _ in range(60):
                            nc.tensor.matmul(pgs[0][:64, :64], warm[:, :], warm[:, :64],
                                             start=True, stop=True)
                    for h in range(HT):
                        st, sp = h == 0, h == HT - 1
                        wgh = (wgA if h < HH else wgB)[:, h % HH, :]
                        wuh = (wuA if h < HH else wuB)[:, h % HH, :]
                        for ci, (s, w) in enumerate(ch):
                            nc.tensor.matmul(pgs[ci][:], wgh,
                                             xg_t[h][:, s:s + w], start=st, stop=sp)
                        for ci, (s, w) in enumerate(ch):
                            nc.tensor.matmul(pus[ci][:], wuh,
                                             xg_t[h][:, s:s + w], start=st, stop=sp)
                    sg = spool.tile([P, C], dt.float32, name="sg", tag="sg")
                    ai = apool.tile([P, C], dt.bfloat16, name=f"a{it}", tag=f"a{it}")
                    for ci, (s, w) in enumerate(ch):
                        nc.scalar.activation(sg[:, s:s + w], pgs[ci][:],
                                             mybir.ActivationFunctionType.Silu)
                        nc.vector.tensor_mul(ai[:, s:s + w], sg[:, s:s + w], pus[ci][:])
                    a_t.append(ai)

            # Phase 2: Y^T[h, t] = sum_i Wd[i, h] * A[i, t].
            prev_wd_ins = anchor_ins
            with tc.tile_pool(name="pp2", bufs=2, space="PSUM") as pp2:
                for ht in range(HT):
                    # wd rides GpSimd; chained (ordering-only) behind the
                    # it=5 weight load so its packets can never cut ahead of
                    # phase-1-critical data in the FIFO queues.
                    wdt, wd_ins = load_wd(ht)
                    if prev_wd_ins is not None:
                        add_dep_helper(wd_ins.ins, prev_wd_ins.ins, sync=False)
                    prev_wd_ins = wd_ins
                    pys = [pp2.tile([P, w], dt.float32, name=f"py{ci}", tag=f"py{ci}")
                           for ci, (s, w) in enumerate(ch)]
                    for i2 in range(IT):
                        st, sp = i2 == 0, i2 == IT - 1
                        for ci, (s, w) in enumerate(ch):
                            nc.tensor.matmul(pys[ci][:], wdt[:, i2, :],
                                             a_t[i2][:, s:s + w], start=st, stop=sp)
                    yo = ypool.tile([P, C], dt.float32, name="yo", tag="yo")
                    for ci, (s, w) in enumerate(ch):
                        nc.vector.tensor_mul(yo[:, s:s + w], wb[:, s:s + w], pys[ci][:])
                    nc.scalar.dma_start(out=yt[ht * P:(ht + 1) * P, :], in_=yo[:])
    nc.compile()
    return nc


def _tile_weight(w, nt_out):
    """[K, N] -> [N/128, 128, K] blocks: out[t, p, k*128+c] = w[k*128+p, t*128+c]."""
    K, N = w.shape
    kt = K // P
    return np.ascontiguousarray(
        w.reshape(kt, P, nt_out, P).transpose(2, 1, 0, 3).reshape(nt_out, P, kt * P)
    )


def kernel(hidden_states, gate_w, w_gate, w_up, w_down, top_k):
    global last_results
    hs = np.ascontiguousarray(np.asarray(hidden_states, dtype=np.float32))
    gw = np.asarray(gate_w, dtype=np.float32)
    wg_all = np.asarray(w_gate, dtype=np.float32)
    wu_all = np.asarray(w_up, dtype=np.float32)
    wd_all = np.asarray(w_down, dtype=np.float32)
    K = int(np.asarray(top_k))
    T = hs.shape[0]
    if K <= 0:
        return np.zeros((T, H), np.float32)

    # ---- router (mirrors the reference numerics in fp32) ----
    logits = hs @ gw.T
    m = logits.max(-1, keepdims=True)
    ex = np.exp(logits - m)
    probs = ex / ex.sum(-1, keepdims=True)
    order = np.argsort(-probs, axis=-1, kind="stable")
    topi = order[:, :K]
    topv = np.take_along_axis(probs, topi, axis=-1)
    topv = topv / topv.sum(-1, keepdims=True)

    # ---- dispatch: gather each expert's tokens ----
    idxs, wvs = [], []
    for e in range(E):
        mask = topi == e
        rows = np.nonzero(mask.any(-1))[0]
        idxs.append(rows)
        wvs.append(topv[mask].astype(np.float32))
    counts = [len(r) for r in idxs]
    C = max(64, ((max(counts) + 1) // 2) * 2)

    nc = _compiled.get(C)
    if nc is None:
        nc = _compiled[C] = _build(C)

    bf16 = ml_dtypes.bfloat16
    in_maps = []
    for e in range(E):
        idx, wv = idxs[e], wvs[e]
        n = len(idx)
        xsel = hs[idx]  # [n, H]
        xg_np = np.zeros((HT, P, C), dtype=bf16)
        xg_np[:, :, :n] = xsel.T.astype(bf16).reshape(HT, P, n)
        xg_np = np.ascontiguousarray(xg_np.transpose(1, 0, 2).reshape(P, HT * C))
        wt_np = np.zeros((P, C), dtype=np.float32)
        wt_np[:, :n] = wv[None, :]
        in_maps.append({
            "xg": xg_np,
            "wt": wt_np,
            "wg": _tile_weight(wg_all[e].astype(bf16), IT),
            "wu": _tile_weight(wu_all[e].astype(bf16), IT),
            "wd": _tile_weight(wd_all[e].astype(bf16), HT),
        })

    from concourse.bass_utils import run_bass_kernel_spmd
    res = run_bass_kernel_spmd(nc, in_maps, core_ids=list(range(E)))
    last_results = res

    # ---- combine: scatter-add per-expert outputs ----
    out = np.zeros((T, H), np.float32)
    for e in range(E):
        idx = idxs[e]
        n = len(idx)
        yt_e = res.results[e]["yt"]  # [H, C] fp32
        out[idx] += yt_e[:, :n].T
    return out



# revision 16
# speedup vs baseline: 1.0518x; 1.0518x over previous
"""DeepSeek-V2 normal MoE layer on 8 Trainium2 NeuronCores.

Expert-parallel sharding: core e holds expert e's weights (cast to bf16).
The router (tiny [T,E] matmul + softmax + top-k) runs on the host in fp32 —
this is the dispatch step of the sharding layer: it decides which token rows
are copied to which core. Each core receives its routed tokens (bf16,
host-packed so every DMA moves contiguous partition rows) plus a broadcast
row of the renormalized top-k combine weights. On device, each core computes
the gated-SiLU expert MLP for its tokens (three 2048/1408-contraction matmul
phases in bf16 with fp32 PSUM accumulation, feature-major layout so no
on-device transposes are needed), applies the combine weight in the fp32
output epilogue, and the host scatter-adds the per-expert outputs back into
the full [T, H] result.

Weights are pre-tiled on the host into [n_tiles, 128, contract*128] blocks
so every DMA moves 4 KiB contiguous per partition (vs 256 B chunks when
slicing the natural [H, I] layout — measured 2x DMA throughput difference).

Per-core capacity C = max tokens routed to any expert (rounded up to 2);
pad token columns are zero and carry combine-weight 0, so their
contribution is exactly zero.
"""

import numpy as np
import ml_dtypes


def _ensure_ntff_hook():
    """This image's antenv package lacks axon_hooks, but concourse's
    run_bass_kernel_spmd unconditionally imports it when BASS_TRACE is set.
    Provide the module (and the ctypes NTFF hook from trn_agent_boot, when
    available) so tracing works instead of crashing. Idempotent; never
    overwrites an existing module."""
    import sys
    import types
    try:
        import antenv  # noqa: F401
    except ImportError:
        return
    if "antenv.axon_hooks" in sys.modules:
        return
    try:
        import antenv.axon_hooks  # noqa: F401
        return
    except ImportError:
        pass
    mod = types.ModuleType("antenv.axon_hooks")
    holder = {"h": None}
    mod.set_axon_ntff_profile_hook = lambda h: holder.__setitem__("h", h)
    mod.get_axon_ntff_profile_hook = lambda: holder.get("h")
    sys.modules["antenv.axon_hooks"] = mod
    import antenv as _a
    _a.axon_hooks = mod
    try:
        from trn_agent_boot.trn_boot import _ntff_profile_via_ctypes
        hook = _ntff_profile_via_ctypes("/opt/axon/libaxon_pjrt.so")
        if hook is not None:
            mod.set_axon_ntff_profile_hook(hook)
    except Exception:
        pass


_ensure_ntff_hook()

H = 2048
I_DIM = 1408
E = 8
P = 128
HT = H // P      # 16
IT = I_DIM // P  # 11

_compiled = {}
last_results = None


def _chunks(C):
    """Token-column chunks of <=512 (one PSUM bank / max moving free dim).
    Near-equal sizes with every chunk >=128 where possible: a matmul
    narrower than ~60 cycles pays the NX-dispatch floor (~25ns at 2.4GHz
    regardless of width), so (426,128) beats (512,42) by ~2.5us/kernel."""
    n = (C + 511) // 512
    if n == 1:
        return [(0, C)]
    out = []
    s = 0
    for i in range(n):
        w = C // n + (1 if i < C % n else 0)
        if i == n - 1:
            w = C - s
        out.append((s, w))
        s += w
    return out


def _build(C):
    import concourse.bacc as bacc
    import concourse.mybir as mybir
    import concourse.tile as tile

    dt = mybir.dt
    nc = bacc.Bacc("TRN2", target_bir_lowering=False)
    # Pre-tiled weight layouts: wg/wu [IT, 128, HT*128], wd [HT, 128, IT*128].
    # Block [t, p, k*128+c] = W[k*128+p, t*128+c] of the natural layout, i.e.
    # partition p of block t holds that block's full contraction row,
    # contiguous in DRAM.
    xg = nc.dram_tensor("xg", [P, HT * C], dt.bfloat16, kind="ExternalInput")
    wt = nc.dram_tensor("wt", [P, C], dt.float32, kind="ExternalInput")
    wg = nc.dram_tensor("wg", [IT, P, HT * P], dt.bfloat16, kind="ExternalInput")
    wu = nc.dram_tensor("wu", [IT, P, HT * P], dt.bfloat16, kind="ExternalInput")
    wd = nc.dram_tensor("wd", [HT, P, IT * P], dt.bfloat16, kind="ExternalInput")
    yt = nc.dram_tensor("yt", [H, C], dt.float32, kind="ExternalOutput")

    ch = _chunks(C)

    with tile.TileContext(nc) as tc:
        with (
            tc.tile_pool(name="xpool", bufs=1) as xpool,
            tc.tile_pool(name="apool", bufs=1) as apool,
            tc.tile_pool(name="wpool", bufs=3) as wpool,
            tc.tile_pool(name="wdpool", bufs=4) as wdpool,
            tc.tile_pool(name="spool", bufs=2) as spool,
            tc.tile_pool(name="ypool", bufs=3) as ypool,
        ):
            from concourse.tile_rust import add_dep_helper

            def load_wd(t):
                w_t = wdpool.tile([P, IT, P], dt.bfloat16, name="wd", tag="wd")
                ins = nc.gpsimd.dma_start(out=w_t[:], in_=wd[t, :, :])
                return w_t, ins

            # The DMA hardware queues are FIFO per queue: whatever triggers
            # first transfers first, and phase-1's front is delivery-bound
            # (~355 GB/s). So the trigger order must equal first-use order:
            # the 8 xg octets on Sync and it=0's wg/wu halves on GpSimd,
            # interleaved to match it=0's per-h g/u consumption. Every LATER
            # weight load is ring-paced (bufs=1 tag rings): iteration it's
            # half can only trigger once it-1's same-ring half has been fully
            # consumed — a real WAR dependency the scheduler must honor,
            # which pins all bulk weight traffic behind the whole xg block
            # in the queues.
            HO = HT // 8
            xq_t, xq_ins = [], []

            def load_xq(q):
                t = xpool.tile([P, HO * C], dt.bfloat16, name=f"xq{q}", tag=f"xq{q}")
                ins = nc.sync.dma_start(out=t[:], in_=xg[:, q * HO * C:(q + 1) * HO * C])
                xq_t.append(t)
                xq_ins.append(ins)

            def load_w_half(src, t, half, tag):
                # bufs=2: iteration it's load ring-waits on it-2's consumers,
                # giving a full iteration (~7.4us) of prefetch margin
                w_t = wpool.tile([P, HT // 2, P], dt.bfloat16, name=f"{tag}{t}",
                                 tag=tag, bufs=2)
                ins = nc.gpsimd.dma_start(
                    out=w_t[:], in_=src[t, :, half * (HT // 2) * P:
                                        (half + 1) * (HT // 2) * P])
                return w_t, ins

            with tc.high_priority():
                load_xq(0)
                wgA, _ = load_w_half(wg, 0, 0, "wgA")
                load_xq(1)
                wuA, _ = load_w_half(wu, 0, 0, "wuA")
                load_xq(2)
                load_xq(3)
                wgB, _ = load_w_half(wg, 0, 1, "wgB")
                load_xq(4)
                wuB, _ = load_w_half(wu, 0, 1, "wuB")
                for q in range(5, 8):
                    load_xq(q)
            wb = xpool.tile([P, C], dt.float32, name="wb", tag="wb")
            wb_ins = nc.sync.dma_start(out=wb[:], in_=wt[:, :])
            # ordering-only edge: wb's trigger sits behind xq7's in the Sync
            # stream so its packets cannot cut ahead of token data
            add_dep_helper(wb_ins.ins, xq_ins[7].ins, sync=False)
            xg_t = [xq_t[h // HO][:, (h % HO) * C:(h % HO + 1) * C]
                    for h in range(HT)]

            # PE warm-up while token DMAs stream: ~6us of tiny matmuls on a
            # zeroed scratch tile releases the HAM clock gate (1.2 -> 2.4 GHz
            # takes ~3.4us of sustained PE activity) before real work lands.
            warm = spool.tile([P, 64], dt.bfloat16, name="warm", tag="warm")
            nc.vector.memset(warm[:], 0.0)

            # Phase 1: A[i, t] = silu(G) * U, feature-major, per 128-row i-tile.
            HH = HT // 2
            a_t = []
            anchor_ins = None
            with tc.tile_pool(name="pp1", bufs=2, space="PSUM") as pp1:
                for it in range(IT):
                    if it != 0:
                        wgA, a_ins = load_w_half(wg, it, 0, "wgA")
                        if it == 1:
                            # it=1's loads have a free ring buffer, so pin
                            # them (real cross-engine dep on xq7's trigger
                            # having executed) behind the token block in the
                            # FIFO queues; it>=2 is ring-paced.
                            add_dep_helper(a_ins.ins, xq_ins[7].ins, sync=True)
                        wuA, b_ins = load_w_half(wu, it, 0, "wuA")
                        wgB, c_ins = load_w_half(wg, it, 1, "wgB")
                        wuB, d_ins = load_w_half(wu, it, 1, "wuB")
                        if it == 1:
                            add_dep_helper(b_ins.ins, a_ins.ins, sync=False)
                            add_dep_helper(c_ins.ins, b_ins.ins, sync=False)
                            add_dep_helper(d_ins.ins, c_ins.ins, sync=False)
                        if it == 5:
                            anchor_ins = a_ins
                    pgs = [pp1.tile([P, w], dt.float32, name=f"pg{ci}", tag=f"pg{ci}",
                                    bufs=2 if ci == 0 else 1)
                           for ci, (s, w) in enumerate(ch)]
                    pus = [pp1.tile([P, w], dt.float32, name=f"pu{ci}", tag=f"pu{ci}",
                                    bufs=2 if ci == 0 else 1)
                           for ci, (s, w) in enumerate(ch)]
                    if it == 0:
                        for _ in range(60):
                            nc.tensor.matmul(pgs[0][:64, :64], warm[:, :], warm[:, :64],
                                             start=True, stop=True)
                    for h in range(HT):
                        st, sp = h == 0, h == HT - 1
                        wgh = (wgA if h < HH else wgB)[:, h % HH, :]
                        wuh = (wuA if h < HH else wuB)[:, h % HH, :]
                        for ci, (s, w) in enumerate(ch):
                            nc.tensor.matmul(pgs[ci][:], wgh,
                                             xg_t[h][:, s:s + w], start=st, stop=sp)
                        for ci, (s, w) in enumerate(ch):
                            nc.tensor.matmul(pus[ci][:], wuh,
                                             xg_t[h][:, s:s + w], start=st, stop=sp)
                    sg = spool.tile([P, C], dt.float32, name="sg", tag="sg")
                    ai = apool.tile([P, C], dt.bfloat16, name=f"a{it}", tag=f"a{it}")
                    for ci, (s, w) in enumerate(ch):
                        nc.scalar.activation(sg[:, s:s + w], pgs[ci][:],
                                             mybir.ActivationFunctionType.Silu)
                        nc.vector.tensor_mul(ai[:, s:s + w], sg[:, s:s + w], pus[ci][:])
                    a_t.append(ai)

            # Phase 2: Y^T[h, t] = sum_i Wd[i, h] * A[i, t].
            prev_wd_ins = anchor_ins
            with tc.tile_pool(name="pp2", bufs=2, space="PSUM") as pp2:
                for ht in range(HT):
                    # wd rides GpSimd; chained (ordering-only) behind the
                    # it=5 weight load so its packets can never cut ahead of
                    # phase-1-critical data in the FIFO queues.
                    wdt, wd_ins = load_wd(ht)
                    if prev_wd_ins is not None:
                        add_dep_helper(wd_ins.ins, prev_wd_ins.ins, sync=False)
                    prev_wd_ins = wd_ins
                    pys = [pp2.tile([P, w], dt.float32, name=f"py{ci}", tag=f"py{ci}")
                           for ci, (s, w) in enumerate(ch)]
                    for i2 in range(IT):
                        st, sp = i2 == 0, i2 == IT - 1
                        for ci, (s, w) in enumerate(ch):
                            nc.tensor.matmul(pys[ci][:], wdt[:, i2, :],
                                             a_t[i2][:, s:s + w], start=st, stop=sp)
                    yo = ypool.tile([P, C], dt.float32, name="yo", tag="yo")
                    for ci, (s, w) in enumerate(ch):
                        nc.vector.tensor_mul(yo[:, s:s + w], wb[:, s:s + w], pys[ci][:])
                    nc.scalar.dma_start(out=yt[ht * P:(ht + 1) * P, :], in_=yo[:])
    nc.compile()
    return nc


def _tile_weight(w, nt_out):
    """[K, N] -> [N/128, 128, K] blocks: out[t, p, k*128+c] = w[k*128+p, t*128+c]."""
    K, N = w.shape
    kt = K // P
    return np.ascontiguousarray(
        w.reshape(kt, P, nt_out, P).transpose(2, 1, 0, 3).reshape(nt_out, P, kt * P)
    )


def kernel(hidden_states, gate_w, w_gate, w_up, w_down, top_k):
    global last_results
    hs = np.ascontiguousarray(np.asarray(hidden_states, dtype=np.float32))
    gw = np.asarray(gate_w, dtype=np.float32)
    wg_all = np.asarray(w_gate, dtype=np.float32)
    wu_all = np.asarray(w_up, dtype=np.float32)
    wd_all = np.asarray(w_down, dtype=np.float32)
    K = int(np.asarray(top_k))
    T = hs.shape[0]
    if K <= 0:
        return np.zeros((T, H), np.float32)

    # ---- router (mirrors the reference numerics in fp32) ----
    logits = hs @ gw.T
    m = logits.max(-1, keepdims=True)
    ex = np.exp(logits - m)
    probs = ex / ex.sum(-1, keepdims=True)
    order = np.argsort(-probs, axis=-1, kind="stable")
    topi = order[:, :K]
    topv = np.take_along_axis(probs, topi, axis=-1)
    topv = topv / topv.sum(-1, keepdims=True)

    # ---- dispatch: gather each expert's tokens ----
    idxs, wvs = [], []
    for e in range(E):
        mask = topi == e
        rows = np.nonzero(mask.any(-1))[0]
        idxs.append(rows)
        wvs.append(topv[mask].astype(np.float32))
    counts = [len(r) for r in idxs]
    C = max(64, ((max(counts) + 1) // 2) * 2)

    nc = _compiled.get(C)
    if nc is None:
        nc = _compiled[C] = _build(C)

    bf16 = ml_dtypes.bfloat16
    in_maps = []
    for e in range(E):
        idx, wv = idxs[e], wvs[e]
        n = len(idx)
        xsel = hs[idx]  # [n, H]
        xg_np = np.zeros((HT, P, C), dtype=bf16)
        xg_np[:, :, :n] = xsel.T.astype(bf16).reshape(HT, P, n)
        xg_np = np.ascontiguousarray(xg_np.transpose(1, 0, 2).reshape(P, HT * C))
        wt_np = np.zeros((P, C), dtype=np.float32)
        wt_np[:, :n] = wv[None, :]
        in_maps.append({
            "xg": xg_np,
            "wt": wt_np,
            "wg": _tile_weight(wg_all[e].astype(bf16), IT),
            "wu": _tile_weight(wu_all[e].astype(bf16), IT),
            "wd": _tile_weight(wd_all[e].astype(bf16), HT),
        })

    from concourse.bass_utils import run_bass_kernel_spmd
    res = run_bass_kernel_spmd(nc, in_maps, core_ids=list(range(E)))
    last_results = res

    # ---- combine: scatter-add per-expert outputs ----
    out = np.zeros((T, H), np.float32)
    for e in range(E):
        idx = idxs[e]
        n = len(idx)
        yt_e = res.results[e]["yt"]  # [H, C] fp32
        out[idx] += yt_e[:, :n].T
    return out

